# revision 14
# baseline (speedup 1.0000x reference)
"""GNN NodeModel kernel for 8 Trainium2 NeuronCores.

Strategy: shard edges by DESTINATION node block (512 nodes), so scatter_mean
is fully core-local (no collectives). Key algebraic fusion: scatter_mean
commutes with the linear maps around it, so the per-edge work is ONLY
MLP1-layer1 (+ReLU); the second MLP1 layer and MLP2's hidden contraction
collapse into one per-node matmul with the host-precomputed product
Wc = W1b @ W2a[9:521]:

  out = relu( Wc^T @ mean_e(relu(W1a^T a_e)) + W2a_xu^T [x;u;1] ) @ W2b + b2b

Per core:
  - edge-parallel L1 (edge-major bf16 matmuls, bias via a constant-1 row
    folded into the A1/W1ax operands)
  - scatter-mean via one-hot S-matrix matmuls into a per-window PSUM
    accumulator (one 2KB bank holds all 4 h-chunks of a 128-node window);
    S is scaled by 1/deg(dest) at build time (tensor_scalar is_equal*dlinv),
    so the whole window evicts with a single copy
  - node-parallel fused MLP2 on the aggregated features
All 8 cores run one shared SPMD program. Blocks are LPT-assigned to cores by
padded edge count; within each block the four 128-node scatter windows are
processed in descending-size order (a pure host-side node permutation, so
the shared slot structure aligns big windows with big windows across cores),
and each rank slot is padded to the max across cores.
"""

import os
import sys

sys.path.insert(0, "/opt/trn_rl_repo")

import numpy as np

import concourse.bass as bass
import concourse.mybir as mybir
import concourse.tile as tile
from concourse import bacc
from concourse.bass_utils import run_bass_kernel_spmd

P = 128          # partitions
H = 512          # hidden width
NBN = 512        # nodes per node-block (MLP2 unit)
SW = 128         # nodes per scatter window
NW = NBN // SW   # scatter windows per node-block
EB = 512         # edges per compute block
NCORES = 8

F32 = mybir.dt.float32
BF16 = mybir.dt.bfloat16
NPBF16 = mybir.dt.np(mybir.dt.bfloat16)
I32 = mybir.dt.int32

LAST_RUN_INFO = {}


def _build_structure(row, n_nodes):
    """Partition node blocks across cores; compute shared slot structure.

    Returns per-core block lists, per-block window orderings (descending
    padded size, shared across cores since sizes are a property of the
    global block), and per-rank window slot capacities C (multiples of 128).
    """
    n_blocks_g = -(-n_nodes // NBN)
    wcnt = np.bincount(row // SW, minlength=NW * n_blocks_g).astype(np.int64)
    wpad = np.maximum(P, ((wcnt + P - 1) // P) * P)
    block_pad = wpad.reshape(-1, NW).sum(axis=1)

    # per-block window order: descending padded size
    worder = np.argsort(-wpad.reshape(-1, NW), axis=1, kind="stable")

    # LPT assignment of global blocks to cores, balancing padded edge counts
    order_desc = np.argsort(-block_pad, kind="stable")
    core_blocks = [[] for _ in range(NCORES)]
    core_tot = np.zeros(NCORES, dtype=np.int64)
    for g in order_desc:
        k = int(np.argmin(core_tot))
        core_blocks[k].append(int(g))
        core_tot[k] += block_pad[g]

    nbk = max(len(bl) for bl in core_blocks)
    # per-rank window capacity = max across cores at that rank slot
    C = np.full(NW * nbk, P, dtype=np.int64)
    for bl in core_blocks:
        for j, g in enumerate(bl):
            for q in range(NW):
                C[NW * j + q] = max(C[NW * j + q], wpad[NW * g + worder[g, q]])
    et = int(C.sum())
    rem = (-et) % EB
    C[-1] += rem
    et += rem
    return core_blocks, worder, nbk, C, et, wcnt


def _build_program(nbk, C, et, trace_sim=False, reps=1):
    """Trace the shared SPMD Bass program for the given slot structure."""
    sub = et // P
    ebk = et // EB
    npad = nbk * NBN

    # sub-tile t -> (window slot jw, first?, last?)
    sub_first = {}
    sub_last = {}
    sub_win = np.empty(sub, dtype=np.int64)
    t = 0
    for jw in range(NW * nbk):
        ns = int(C[jw]) // P
        for s in range(ns):
            sub_win[t] = jw
            if s == 0:
                sub_first[t] = True
            if s == ns - 1:
                sub_last[t] = True
            t += 1
    assert t == sub

    nc = bacc.Bacc("TRN2", target_bir_lowering=False, debug=False)
    A0 = nc.declare_dram_parameter("a0", [P, 4, et], BF16, isOutput=False)
    A1 = nc.declare_dram_parameter("a1", [10, et], BF16, isOutput=False)
    DLOC = nc.declare_dram_parameter("dloc", [P, sub], F32, isOutput=False)
    DLINV = nc.declare_dram_parameter("dlinv", [P, sub], F32, isOutput=False)
    XU = nc.declare_dram_parameter("xu", [26, npad], BF16, isOutput=False)
    W1AE = nc.declare_dram_parameter("w1ae", [P, 4, H], BF16, isOutput=False)
    W1AX = nc.declare_dram_parameter("w1ax", [10, H], BF16, isOutput=False)
    WC = nc.declare_dram_parameter("wc", [P, 4, H], BF16, isOutput=False)
    W2AX = nc.declare_dram_parameter("w2ax", [26, H], BF16, isOutput=False)
    W2B = nc.declare_dram_parameter("w2b", [P, 4], BF16, isOutput=False)
    OUT = nc.declare_dram_parameter("out", [1, npad], F32, isOutput=True)

    with tile.TileContext(nc, trace_sim=trace_sim) as tc:
        with (
            tc.tile_pool(name="wpool", bufs=1) as wpool,
            tc.tile_pool(name="apool", bufs=3) as apool,
            tc.tile_pool(name="hpool", bufs=3) as hpool,
            tc.tile_pool(name="spool", bufs=4) as spool,
            tc.tile_pool(name="ztpool", bufs=2) as ztpool,
            tc.tile_pool(name="ttpool", bufs=4) as ttpool,
            tc.tile_pool(name="mmps", bufs=3, space="PSUM") as mmps,
            tc.tile_pool(name="aggps", bufs=1, space="PSUM") as aggps,
            tc.tile_pool(name="outps", bufs=1, space="PSUM") as outps,
        ):
            # ---- constants / weights ----
            w1ae = wpool.tile([P, 4, H], BF16)
            nc.sync.dma_start(w1ae[:], W1AE[:])
            w1ax = wpool.tile([10, H], BF16)
            nc.sync.dma_start(w1ax[:], W1AX[:])
            wc = wpool.tile([P, 4, H], BF16)
            nc.sync.dma_start(wc[:], WC[:])
            w2ax = wpool.tile([26, H], BF16)
            nc.sync.dma_start(w2ax[:], W2AX[:])
            w2b = wpool.tile([P, 4], BF16)
            nc.sync.dma_start(w2b[:], W2B[:])
            dloc = wpool.tile([P, sub], F32)
            nc.sync.dma_start(dloc[:], DLOC[:])
            dlinv = wpool.tile([P, sub], F32)
            nc.sync.dma_start(dlinv[:], DLINV[:])

            iota_i = wpool.tile([P, SW], I32)
            nc.gpsimd.iota(iota_i[:], pattern=[[1, SW]], base=0, channel_multiplier=0)
            iota_f = wpool.tile([P, SW], F32)
            nc.vector.tensor_copy(iota_f[:], iota_i[:])

            out_row = wpool.tile([1, npad], F32)

            cur_agg = [None]  # live agg psum banks of the in-flight window
            cur_zt = [None]   # agg staging tile of the in-flight node block
            cur_xu = [None]   # prefetched xu tile of the in-flight node block

            def mlp2(j, zt):
                xu = apool.tile([26, NBN], BF16, name="xu")
                nc.sync.dma_start(xu[:], XU[:, j * NBN:(j + 1) * NBN])
                tts = []
                for m in range(4):
                    pst = mmps.tile([P, NBN], F32, tag="mm")
                    for k in range(4):
                        nc.tensor.matmul(
                            pst[:], wc[:, k, m * P:(m + 1) * P], zt[:, k, :],
                            start=(k == 0), stop=False,
                        )
                    nc.tensor.matmul(
                        pst[:], w2ax[:, m * P:(m + 1) * P], xu[:],
                        start=False, stop=True,
                    )
                    tt = ttpool.tile([P, NBN], BF16, name=f"tt{m}")
                    if m < 2:
                        nc.scalar.activation(
                            tt[:], pst[:], mybir.ActivationFunctionType.Relu,
                        )
                    else:
                        nc.vector.tensor_scalar(
                            out=tt[:], in0=pst[:], scalar1=0.0, scalar2=None,
                            op0=mybir.AluOpType.max,
                        )
                    tts.append(tt)
                ops = outps.tile([1, NBN], F32, tag="outps")
                for k in range(4):
                    nc.tensor.matmul(
                        ops[:], w2b[:, k:k + 1], tts[k][:],
                        start=(k == 0), stop=(k == 3),
                    )
                nc.vector.tensor_copy(out_row[0:1, j * NBN:(j + 1) * NBN], ops[:])

            # ---- main loop over edge blocks ----
            for _rep in range(reps):
              for b in range(ebk):
                a0 = apool.tile([P, 4, EB], BF16, name="a0")
                nc.sync.dma_start(a0[:], A0[:, :, b * EB:(b + 1) * EB])
                a1 = apool.tile([10, EB], BF16, name="a1")
                nc.sync.dma_start(a1[:], A1[:, b * EB:(b + 1) * EB])

                for es in range(4):
                    t = b * 4 + es
                    # L1: h1r = relu(W1a^T a_e + b1a), edge-major [128e, H]
                    ps = mmps.tile([P, H], F32, tag="mm")
                    for k in range(4):
                        nc.tensor.matmul(
                            ps[:], a0[:, k, es * P:(es + 1) * P], w1ae[:, k, :],
                            start=(k == 0), stop=False,
                        )
                    nc.tensor.matmul(
                        ps[:], a1[:, es * P:(es + 1) * P], w1ax[:],
                        start=False, stop=True,
                    )
                    h1r = hpool.tile([P, H], BF16, name="h1r")
                    nc.scalar.activation(
                        h1r[:], ps[:], mybir.ActivationFunctionType.Relu,
                    )

                    # scatter-mean: agg[:, m, :] += h1r[:, m-chunk]^T @ S
                    # S one-hot scaled by 1/deg(dest) (mean folded in)
                    jw = int(sub_win[t])
                    j, q = jw // NW, jw % NW
                    s_t = spool.tile([P, SW], BF16, name="s")
                    nc.vector.tensor_scalar(
                        out=s_t[:], in0=iota_f[:], scalar1=dloc[:, t:t + 1],
                        scalar2=dlinv[:, t:t + 1],
                        op0=mybir.AluOpType.is_equal,
                        op1=mybir.AluOpType.mult,
                    )
                    first = sub_first.get(t, False)
                    last = sub_last.get(t, False)
                    if first:
                        cur_agg[0] = [
                            aggps.tile([P, SW], F32, tag=f"agg{m}", name=f"agg{m}")
                            for m in range(4)
                        ]
                    agg = cur_agg[0]
                    for m in range(4):
                        nc.tensor.matmul(
                            agg[m][:], h1r[:, m * P:(m + 1) * P],
                            s_t[:], start=first, stop=last, skip_group_check=True,
                        )
                    if last:
                        # evict window (mean already applied via S scaling);
                        # split copies across DVE and Act to balance engines
                        if q == 0:
                            cur_zt[0] = ztpool.tile([P, 4, NBN], BF16, name="zt")
                        zt = cur_zt[0]
                        for m in range(4):
                            dst = zt[:, m, q * SW:(q + 1) * SW]
                            if m < 2:
                                nc.vector.tensor_copy(dst, agg[m][:])
                            else:
                                nc.scalar.activation(
                                    dst, agg[m][:],
                                    mybir.ActivationFunctionType.Copy,
                                )
                        if q == NW - 1:
                            mlp2(j, zt)

            nc.sync.dma_start(OUT[:], out_row[:])

    if not trace_sim:
        nc.compile()
    return nc


def kernel(**inputs):
    x = np.ascontiguousarray(np.asarray(inputs["x"], dtype=np.float32))
    edge_index = np.asarray(inputs["edge_index"], dtype=np.int64)
    edge_attr = np.ascontiguousarray(np.asarray(inputs["edge_attr"], dtype=np.float32))
    u = np.asarray(inputs["u"], dtype=np.float32)
    batch = np.asarray(inputs["batch"], dtype=np.int64)
    W1a = np.asarray(inputs["W1a"], dtype=np.float32)
    b1a = np.asarray(inputs["b1a"], dtype=np.float32)
    W1b = np.asarray(inputs["W1b"], dtype=np.float32)
    b1b = np.asarray(inputs["b1b"], dtype=np.float32)
    W2a = np.asarray(inputs["W2a"], dtype=np.float32)
    b2a = np.asarray(inputs["b2a"], dtype=np.float32)
    W2b = np.asarray(inputs["W2b"], dtype=np.float32)
    b2b = np.asarray(inputs["b2b"], dtype=np.float32)

    n_nodes = x.shape[0]
    row, col = edge_index[0], edge_index[1]

    cnt = np.bincount(row, minlength=n_nodes)
    inv = (1.0 / np.maximum(cnt, 1)).astype(np.float32)

    core_blocks, worder, nbk, C, et, wcnt = _build_structure(row, n_nodes)
    sub = et // P
    npad = nbk * NBN
    Cstart = np.concatenate([[0], np.cumsum(C)])

    nc = _build_program(nbk, C, et)

    # ---- per-core shards ----
    order = np.argsort(row, kind="stable")
    wstart = np.concatenate([[0], np.cumsum(wcnt)])

    # weights (shared by all cores)
    W1a_e = np.ascontiguousarray(
        W1a[9:521].reshape(4, P, H).transpose(1, 0, 2).astype(NPBF16))
    W1a_x = np.ascontiguousarray(
        np.vstack([W1a[0:9], b1a[None, :]]).astype(NPBF16))
    Wc = W1b @ W2a[9:521]                        # [512, 512]
    bc = b1b @ W2a[9:521] + b2a                  # [512]
    Wc_r = np.ascontiguousarray(
        Wc.reshape(4, P, H).transpose(1, 0, 2).astype(NPBF16))
    W2a_x = np.ascontiguousarray(
        np.vstack([W2a[0:9], W2a[521:537], bc[None, :]]).astype(NPBF16))
    W2b_r = np.ascontiguousarray(W2b[:, 0].reshape(4, P).T.astype(NPBF16))

    xT = x.T  # [9, N]
    uT_b = u[batch].T  # [16, N]

    in_maps = []
    core_slot_blocks = []
    for k in range(NCORES):
        blocks = core_blocks[k] + [-1] * (nbk - len(core_blocks[k]))
        core_slot_blocks.append(blocks)
        eidx = np.full(et, -1, dtype=np.int64)
        # node base of each slot window (for dloc); -1 for padding slots
        slot_base = np.full(NW * nbk, -1, dtype=np.int64)
        for j, g in enumerate(blocks):
            if g >= 0:
                for q in range(NW):
                    gw = NW * g + int(worder[g, q])
                    ne = int(wcnt[gw])
                    s0 = Cstart[NW * j + q]
                    eidx[s0:s0 + ne] = order[wstart[gw]:wstart[gw] + ne]
                    slot_base[NW * j + q] = gw * SW
        valid = eidx >= 0
        e_safe = np.where(valid, eidx, 0)

        ea = edge_attr[e_safe]  # [et, 512]
        A0 = np.ascontiguousarray(
            ea.T.reshape(4, P, et).transpose(1, 0, 2).astype(NPBF16))
        A1 = np.ascontiguousarray(
            np.vstack([x[col[e_safe]].T,
                       np.ones((1, et), np.float32)]).astype(NPBF16))

        dest = row[e_safe]
        # dest-local index within the slot's scatter window
        base_of_slot = np.repeat(slot_base, C)
        dl = np.where(valid & (base_of_slot >= 0),
                      (dest - base_of_slot).astype(np.float32), -1.0)
        dloc_a = np.ascontiguousarray(
            dl.astype(np.float32).reshape(sub, P).T)  # [128, sub]
        dli = np.where(valid, inv[dest], 0.0).astype(np.float32)
        dlinv_a = np.ascontiguousarray(dli.reshape(sub, P).T)  # [128, sub]

        xu_a = np.zeros((26, npad), dtype=np.float32)
        xu_a[25, :] = 1.0
        for j, g in enumerate(blocks):
            if g < 0:
                continue
            for q in range(NW):
                lo = (NW * g + int(worder[g, q])) * SW
                hi = min(lo + SW, n_nodes)
                w = hi - lo
                if w <= 0:
                    continue
                c0 = j * NBN + q * SW
                xu_a[0:9, c0:c0 + w] = xT[:, lo:hi]
                xu_a[9:25, c0:c0 + w] = uT_b[:, lo:hi]

        in_maps.append({
            "a0": A0, "a1": A1, "dloc": dloc_a, "dlinv": dlinv_a,
            "xu": np.ascontiguousarray(xu_a.astype(NPBF16)),
            "w1ae": W1a_e, "w1ax": W1a_x, "wc": Wc_r,
            "w2ax": W2a_x, "w2b": W2b_r,
        })

    res = run_bass_kernel_spmd(nc, in_maps, core_ids=list(range(NCORES)), trace=False)
    LAST_RUN_INFO.clear()
    LAST_RUN_INFO.update({
        "exec_time_ns": res.exec_time_ns,
        "nc": nc,
        "in_maps": in_maps,
        "structure": (nbk, C, et),
    })

    out_full = np.zeros(n_nodes, dtype=np.float32)
    for k in range(NCORES):
        o = res.results[k]["out"][0]
        for j, g in enumerate(core_slot_blocks[k]):
            if g < 0:
                continue
            for q in range(NW):
                lo = (NW * g + int(worder[g, q])) * SW
                hi = min(lo + SW, n_nodes)
                w = hi - lo
                if w <= 0:
                    continue
                c0 = j * NBN + q * SW
                out_full[lo:hi] = o[c0:c0 + w]

    result = out_full[:, None] + b2b[None, :] if b2b.ndim == 1 else out_full[:, None] + b2b
    return result.astype(np.float32)


def _bench_build(nc, in_maps, reps):
    """Build a jitted SPMD executable running the NEFF `reps` times back-to-back."""
    import jax
    import jax.numpy as jnp
    from jax.sharding import Mesh, PartitionSpec
    from jax.experimental.shard_map import shard_map

    from concourse import bass2jax
    from concourse import mybir as _mybir

    bass2jax.install_neuronx_cc_hook()
    partition_name = nc.partition_id_tensor.name if nc.partition_id_tensor else None

    in_names, out_names, out_avals, zero_outs = [], [], [], []
    for alloc in nc.m.functions[0].allocations:
        if not isinstance(alloc, _mybir.MemoryLocationSet):
            continue
        name = alloc.memorylocations[0].name
        if alloc.kind == "ExternalInput":
            if name != partition_name:
                in_names.append(name)
        elif alloc.kind == "ExternalOutput":
            shape = tuple(alloc.tensor_shape)
            dtype = _mybir.dt.np(alloc.dtype)
            out_names.append(name)
            out_avals.append(jax.core.ShapedArray(shape, dtype))
            zero_outs.append(np.zeros(shape, dtype))
    n_params = len(in_names)
    chain_idx = in_names.index("dloc") if "dloc" in in_names else 0
    all_in_names = in_names + out_names
    if partition_name is not None:
        all_in_names.append(partition_name)

    bind_kw = dict(
        out_avals=tuple(out_avals),
        in_names=tuple(all_in_names),
        out_names=tuple(out_names),
        lowering_input_output_aliases=(),
        sim_require_finite=True,
        sim_require_nnan=True,
        nc=nc,
    )

    assert reps == 1

    def _body(*args):
        operands = list(args)
        if partition_name is not None:
            operands.append(bass2jax.partition_id_tensor())
        outs = bass2jax._bass_exec_p.bind(*operands, **bind_kw)
        return tuple(outs)

    n_cores = len(in_maps)
    devices = jax.devices()[:n_cores]
    mesh = Mesh(np.asarray(devices), ("core",))
    in_specs = (PartitionSpec("core"),) * (n_params + len(out_names))
    out_specs = (PartitionSpec("core"),) * len(out_names)
    fn = jax.jit(
        shard_map(_body, mesh=mesh, in_specs=in_specs, out_specs=out_specs,
                  check_rep=False),
        keep_unused=True,
    )
    concat_in = [
        np.concatenate([np.asarray(in_maps[c][nm]) for c in range(n_cores)], axis=0)
        for nm in in_names
    ] + [np.concatenate([z] * n_cores, axis=0) for z in zero_outs]
    sharding = jax.sharding.NamedSharding(mesh, PartitionSpec("core"))
    args = [jax.device_put(a, sharding) for a in concat_in]
    return fn, args


def _pipe_time(fn, args, n_pipe, iters):
    import time

    fn(*args)[0].block_until_ready()  # warm
    best = float("inf")
    for _ in range(iters):
        t0 = time.perf_counter()
        outs = [fn(*args) for _ in range(n_pipe)]
        outs[-1][0].block_until_ready()
        best = min(best, (time.perf_counter() - t0) / n_pipe)
    return best


def bench(r_lo=16, r_hi=32, n_pipe=24, iters=6):
    """Per-NEFF-body exec time: marginal cost between r_hi-x and r_lo-x
    replicated bodies, both deep enough that device execution (not dispatch
    RPC) is the pipeline bottleneck."""
    in_maps = LAST_RUN_INFO["in_maps"]
    st = LAST_RUN_INFO["structure"]

    times = {}
    for r in (r_lo, r_hi):
        ncR = _build_program(*st, reps=r)
        fnR, argsR = _bench_build(ncR, in_maps, 1)
        times[r] = _pipe_time(fnR, argsR, n_pipe, iters)
    exec_ns = (times[r_hi] - times[r_lo]) / (r_hi - r_lo) * 1e9
    LAST_RUN_INFO["exec_time_ns"] = exec_ns
    LAST_RUN_INFO["bench_detail"] = {f"t{r}_ms": f"{t * 1e3:.2f}" for r, t in times.items()}
    return exec_ns
